# revision 19
# baseline (speedup 1.0000x reference)
"""Trainium2 Bass kernel for the 2-layer GraphSAGE encoder (mean aggregation).

Computation (see reference):
  h   = relu(mean_agg(relu(x)[src] by dst) @ W_l1 + b_l1 + x @ W_r1)
  out =      mean_agg(h[src]       by dst) @ W_l2 + b_l2 + h @ W_r2

Distribution: edges are partitioned across the 8 cores by destination
shard (12500 nodes each).  Within a core, edges are grouped by
(dst window of 128 nodes, src bank of 4) and padded to chunks of 128.

Each core receives only its own raw x shard (bf16); the relu'd gather
table is assembled on device via 4 AllGathers into Shared DRAM, and
the transposed root (xT) is derived from the same shard with tensor-
engine transposes, minimizing per-dispatch input marshaling (which
dominates the dispatch wall through this runtime).  Gather indices are
shipped unreplicated and fanned 16->128 partitions with 8 DMAs; the
output is written bf16 and upcast on host.  ALL inputs (x shard, index
/selector streams, weights) are packed into a single bf16 DRAM
parameter and unpacked on device with AP bitcasts, so the timed
dispatch carries exactly one input and one output buffer.

Messages are fetched with batched dma_gather (bf16 tables, 256B rows)
spread across the 4 SWDGE queues (one per bank); aggregation contracts
gathered chunks against value-scaled one-hot selectors on the tensor
engine with PSUM accumulation per window, producing transposed
aggregates directly.  Per-edge 1/deg values are folded into the
selectors (no count/mean division on device).  Selectors are built
with batched DVE tensor_tensor ops (is_equal/mult vs broadcast APs)
which never contend with gpsimd SWDGE descriptor generation; all
PSUM->SBUF moves run on the scalar engine.  Roots stay SBUF-resident
transposed (xT for layer 1, hT written during layer 1 for layer 2).
Between layers, h is published as bf16 in 4 quarter-pieces via 4
AllGathers that overlap layer-1 compute.
"""
import os
import sys

sys.path.insert(0, "/opt/trn_rl_repo")

import numpy as np
import ml_dtypes

import concourse.bacc as bacc
import concourse.tile as tile
from concourse import bass, mybir
from concourse.bass_utils import run_bass_kernel_spmd
from concourse.masks import make_identity

F32 = mybir.dt.float32
BF16 = mybir.dt.bfloat16
I16 = mybir.dt.int16
BF = ml_dtypes.bfloat16

P = 128          # partition width / chunk size / feature dim
D = 128          # feature dim
NCORES = 8
NQ = 4           # src banks (= table quarters; int16 index limit)
PAD_DOFF = 300.0  # dstoff value for pad slots (matches no iota lane)

LAST_EXEC_NS = None
LAST_RESULTS = None
LAST_NC = None
LAST_IN_MAPS = None


class Cfg:
    def __init__(self, n_nodes, n_edges):
        assert n_nodes % (NCORES * NQ) == 0
        self.N = n_nodes
        self.E = n_edges
        self.NSH = n_nodes // NCORES          # nodes per dst shard
        self.QR = self.NSH // NQ              # real rows per quarter
        self.WQ = -(-self.QR // P)            # windows per quarter
        self.QP = self.WQ * P                 # padded rows per quarter
        self.W = NQ * self.WQ                 # windows per core
        self.SGW = 5 if self.WQ % 5 == 0 else 1   # windows per super-group
        assert self.WQ % self.SGW == 0
        self.NSG = self.W // self.SGW
        self.BR = NCORES * self.QP            # rows per bank
        assert self.BR - 1 <= 32767, "bank exceeds int16 index range"
        self.VPAD = NQ * self.BR              # padded table rows


def _map_nodes(cfg, node):
    """Map raw node ids -> (bank, in-bank row) of the quarter-major table."""
    c = node // cfg.NSH
    local = node % cfg.NSH
    q = np.minimum(local // cfg.QR, NQ - 1)
    r = local - q * cfg.QR
    return q, c * cfg.QP + r


def _host_prep(cfg, x, edge_index):
    """Build per-core gather-index / dstoff / value streams and layouts."""
    src = np.asarray(edge_index[0], dtype=np.int64)
    dst = np.asarray(edge_index[1], dtype=np.int64)
    E = src.shape[0]

    core = dst // cfg.NSH
    dl = dst % cfg.NSH
    qd = np.minimum(dl // cfg.QR, NQ - 1)
    rd = dl - qd * cfg.QR
    win = qd * cfg.WQ + rd // P            # window within core
    doff = rd % P                          # one-hot lane within window
    bank, idx16 = _map_nodes(cfg, src)

    # per-edge aggregation weight: 1/max(indegree(dst), 1)
    cnt = np.bincount(dst, minlength=cfg.N).astype(np.float64)
    val_edge = (1.0 / np.maximum(cnt, 1.0))[dst]

    # counts per (core, window, bank)
    key = ((core * cfg.W + win) * NQ + bank).astype(np.int64)
    counts = np.bincount(key, minlength=NCORES * cfg.W * NQ).reshape(
        NCORES, cfg.W, NQ
    )
    kwb = -(-counts.max(axis=0) // P)      # [W, NQ] chunks, shared layout
    kwb[:, 0] = np.maximum(kwb[:, 0], 1)   # every window needs >=1 chunk

    # stream order: for sg: for b: for w in sg: for k in K_wb[w,b]
    order = []                              # (w, b) in stream order
    for s in range(cfg.NSG):
        ws = range(s * cfg.SGW, (s + 1) * cfg.SGW)
        for b in range(NQ):
            for w in ws:
                order.append((w, b))
    chunk_base = {}                         # (w,b) -> first chunk idx in stream
    nch = 0
    for (w, b) in order:
        chunk_base[(w, b)] = nch
        nch += int(kwb[w, b])
    total_slots = nch * P

    # slot position of every edge within its core's stream
    edge_sort = np.lexsort((src, key))      # group by (core, win, bank)
    ks = key[edge_sort]
    group_start = np.searchsorted(ks, np.arange(NCORES * cfg.W * NQ), side="left")
    rank_within = np.arange(E) - group_start[ks]
    cw = ks // NQ
    wb_w = (cw % cfg.W).astype(np.int64)
    wb_b = (ks % NQ).astype(np.int64)
    base_arr = np.zeros((cfg.W, NQ), dtype=np.int64)
    for (w, b), cb in chunk_base.items():
        base_arr[w, b] = cb * P
    slot = base_arr[wb_w, wb_b] + rank_within
    edge_core = (ks // (cfg.W * NQ)).astype(np.int64)

    idx_streams = np.zeros((NCORES, total_slots), dtype=np.int16)
    doff_streams = np.full((NCORES, total_slots), -1, dtype=np.int8)
    cnt_streams = np.ones((NCORES, total_slots), dtype=np.int8)
    idx_streams[edge_core, slot] = idx16[edge_sort].astype(np.int16)
    doff_streams[edge_core, slot] = doff[edge_sort].astype(np.int8)
    cnt_clip = np.maximum(cnt, 1.0)
    assert cnt_clip.max() <= 127, "in-degree exceeds int8 packing range"
    cnt_streams[edge_core, slot] = cnt_clip[dst][edge_sort].astype(np.int8)

    # idx wrap16 layout [16, total/16] (device fans out to 128 partitions);
    # doff/cnt [128, nch] chunk-major int8 (device casts / reciprocates)
    idxw16 = np.ascontiguousarray(
        idx_streams.reshape(NCORES, total_slots // 16, 16).transpose(0, 2, 1)
    )
    doff8 = np.ascontiguousarray(
        doff_streams.reshape(NCORES, nch, P).transpose(0, 2, 1)
    )
    cnt8 = np.ascontiguousarray(
        cnt_streams.reshape(NCORES, nch, P).transpose(0, 2, 1)
    )

    # per-core raw x shard, padded quarter-major local rows, bf16
    xsh = np.zeros((NCORES, NQ * cfg.QP, D), dtype=BF)
    nodes = np.arange(cfg.N, dtype=np.int64)
    c_all = nodes // cfg.NSH
    local = nodes % cfg.NSH
    q_all = np.minimum(local // cfg.QR, NQ - 1)
    r_all = local - q_all * cfg.QR
    xsh[c_all, q_all * cfg.QP + r_all] = x.astype(BF)

    return dict(
        kwb=kwb,
        chunk_base=chunk_base,
        order=order,
        nch=nch,
        idxw16=idxw16,
        doff8=doff8,
        cnt8=cnt8,
        xsh=xsh,
    )


def _build_program(cfg, kwb, nch, repeat=1):
    """Emit the SPMD Bass program. kwb: [W, NQ] chunk counts (static)."""
    nc = bacc.Bacc(
        None, target_bir_lowering=False, debug=False, num_swdge_queues=4
    )
    kwb = np.asarray(kwb)

    # single packed input: rows of 128 bf16 elements
    #   [0, R0)              x shard (row layout)
    #   [R0, R0+nch)         gather idx bits ([16, nch*8] i16 row-major)
    #   next nch2/2          doff  ([128, nch2] int8 row-major, -1 = pad)
    #   next nch2/2          cnt   ([128, nch2] int8 row-major, >= 1)
    #   next 384             wr1|wr2|iota   ([128, 384] bf16 row-major)
    #   next 516             wl1|wl2|bl1|bl2 ([128, 258] f32 row-major bits)
    nch2 = nch + (nch & 1)
    R0 = NQ * cfg.QP
    r_idx = R0
    r_doff = R0 + nch
    r_cnt = r_doff + nch2 // 2
    r_wbf = r_cnt + nch2 // 2
    r_wf = r_wbf + 384
    rows_total = r_wf + 516
    pk_t = nc.declare_dram_parameter("xmy", [rows_total, D], BF16, isOutput=False)
    out_t = nc.declare_dram_parameter("out", [NQ * cfg.QP, D], BF16, isOutput=True)

    def region(row0, nrows, spec, **kw):
        return (
            pk_t[row0 : row0 + nrows, :]
            .rearrange("r c -> (r c)")
            .rearrange(spec, **kw)
        )

    xsh_t = pk_t  # rows [0, R0) are the x shard

    # chunk index in the stream for (w, b, k)
    base_arr = np.zeros((cfg.W, NQ), dtype=np.int64)
    nch_chk = 0
    for s in range(cfg.NSG):
        ws = range(s * cfg.SGW, (s + 1) * cfg.SGW)
        for b in range(NQ):
            for w in ws:
                base_arr[w, b] = nch_chk
                nch_chk += int(kwb[w, b])
    assert nch_chk == nch

    # per-window (bank, k) sequence for start/stop flags
    win_seq = []
    for w in range(cfg.W):
        seq = [(b, k) for b in range(NQ) for k in range(int(kwb[w, b]))]
        win_seq.append(seq)

    assert cfg.SGW <= 5, "psum banks: need one per open window group"

    qload = [0] * 4  # cumulative chunks per SWDGE queue (greedy balance)

    with tile.TileContext(nc, trace_sim=bool(os.environ.get("GNN_TRACE_SIM"))) as tc:
        with (
            tc.tile_pool(name="const", bufs=1) as cp,
            tc.tile_pool(name="gather", bufs=6) as gp,
            tc.tile_pool(name="onehot", bufs=4) as op_,
            tc.tile_pool(name="wstage", bufs=3) as wp,
            tc.tile_pool(name="mps", bufs=1, space="PSUM") as mpp,
            tc.tile_pool(name="wps", bufs=2, space="PSUM") as wpp,
            tc.tile_pool(name="dram", bufs=1, space="DRAM") as dp,
        ):
            ident_bf = cp.tile([P, P], BF16)
            make_identity(nc, ident_bf[:])
            wbf_s = cp.tile([P, 3 * D], BF16)
            nc.sync.dma_start(
                wbf_s[:], region(r_wbf, 384, "(p c) -> p c", p=P)
            )
            wr1 = wbf_s[:, 0:D]
            wr2 = wbf_s[:, D : 2 * D]
            iota_s = wbf_s[:, 2 * D : 3 * D]
            wf_s = cp.tile([P, 258], F32)
            nc.sync.dma_start(
                wf_s[:],
                region(r_wf, 516, "(p c) -> p c", p=P).bitcast(F32),
            )
            wl1 = wf_s[:, 0:D]
            wl2 = wf_s[:, D : 2 * D]
            bl1 = wf_s[:, 2 * D : 2 * D + 1]
            bl2 = wf_s[:, 2 * D + 1 : 2 * D + 2]
            idx_src = region(r_idx, nch, "(a b) -> a b", a=16).bitcast(I16)
            idx_s = cp.tile([P, (nch * P) // 16], I16)
            for k in range(8):
                nc.sync.dma_start(idx_s[k * 16 : (k + 1) * 16, :], idx_src)
            I8 = mybir.dt.int8
            d8 = cp.tile([P, nch2], I8)
            nc.sync.dma_start(
                d8[:],
                region(r_doff, nch2 // 2, "(p c) -> p c", p=P).bitcast(I8),
            )
            c8 = cp.tile([P, nch2], I8)
            nc.sync.dma_start(
                c8[:],
                region(r_cnt, nch2 // 2, "(p c) -> p c", p=P).bitcast(I8),
            )
            doff_s = cp.tile([P, nch], BF16)
            nc.vector.tensor_copy(doff_s[:], d8[:, 0:nch])
            valf = cp.tile([P, nch], F32)
            nc.vector.tensor_copy(valf[:], c8[:, 0:nch])
            nc.vector.reciprocal(valf[:], valf[:])
            val_s = cp.tile([P, nch], BF16)
            nc.vector.tensor_copy(val_s[:], valf[:])
            xT_s = cp.tile([P, NQ * cfg.QP], BF16)
            hT_s = cp.tile([P, NQ * cfg.QP], BF16)
            tc.strict_bb_all_engine_barrier()

            for rep in range(repeat):
                xrelu = [
                    dp.tile([cfg.QP, D], BF16, name=f"xrelu{q}_{rep}")
                    for q in range(NQ)
                ]
                xtbl = [
                    dp.tile(
                        [cfg.BR, D], BF16, addr_space="Shared",
                        name=f"xtbl{q}_{rep}",
                    )
                    for q in range(NQ)
                ]
                hpub = [
                    dp.tile([cfg.QP, D], BF16, name=f"hpub{q}_{rep}")
                    for q in range(NQ)
                ]
                htbl = [
                    dp.tile(
                        [cfg.BR, D], BF16, addr_space="Shared",
                        name=f"htbl{q}_{rep}",
                    )
                    for q in range(NQ)
                ]

                # derive relu(x) table pieces + resident xT from the raw shard
                for q in range(NQ):
                    for wq in range(cfg.WQ):
                        w = q * cfg.WQ + wq
                        xw = wp.tile([P, P], BF16, tag="xw")
                        nc.sync.dma_start(
                            xw[:], xsh_t[w * P : (w + 1) * P, :]
                        )
                        xrl = wp.tile([P, P], BF16, tag="xrl")
                        nc.scalar.activation(
                            xrl[:], xw[:], mybir.ActivationFunctionType.Relu
                        )
                        nc.sync.dma_start(
                            xrelu[q][wq * P : (wq + 1) * P, :], xrl[:]
                        )
                        tpx = wpp.tile([P, P], BF16, tag="tps", space="PSUM")
                        nc.tensor.transpose(
                            out=tpx[:], in_=xw[:], identity=ident_bf[:]
                        )
                        nc.scalar.activation(
                            xT_s[:, w * P : (w + 1) * P], tpx[:],
                            mybir.ActivationFunctionType.Identity,
                        )
                    nc.gpsimd.collective_compute(
                        "AllGather",
                        mybir.AluOpType.bypass,
                        replica_groups=[list(range(NCORES))],
                        ins=[xrelu[q][:].opt()],
                        outs=[xtbl[q][:].opt()],
                    )

                for layer in (1, 2):
                    if layer == 1:
                        tables = [xtbl[b][:, :] for b in range(NQ)]
                        rootT, wl, wr, bl = xT_s, wl1, wr1, bl1  # APs
                    else:
                        tables = [htbl[b][:, :] for b in range(NQ)]
                        rootT, wl, wr, bl = hT_s, wl2, wr2, bl2  # APs

                    for s in range(cfg.NSG):
                        ws = list(range(s * cfg.SGW, (s + 1) * cfg.SGW))
                        # one psum tile per window: transposed mean [feat, dst]
                        wt = [
                            mpp.tile([P, P], F32, tag=f"win{wi}", space="PSUM",
                                     name=f"winps{wi}")
                            for wi in range(len(ws))
                        ]

                        for b in range(NQ):
                            cb0 = base_arr[ws[0], b]
                            csb = sum(int(kwb[w, b]) for w in ws)
                            if csb == 0:
                                continue
                            gb = gp.tile([P, csb * P], BF16, tag="gb")
                            gb3 = gb[:].rearrange("p (g e) -> p g e", e=P)
                            qn = min(range(4), key=lambda i: qload[i])
                            qload[qn] += csb
                            nc.gpsimd.dma_gather(
                                out_ap=gb3[:, :, :],
                                in_ap=tables[b],
                                idxs_ap=idx_s[:, cb0 * 8 : (cb0 + csb) * 8],
                                num_idxs=csb * P,
                                num_idxs_reg=csb * P,
                                elem_size=D,
                                single_packet=False,
                                queue_num=qn,
                            )
                            # value-scaled one-hot selectors, batched build
                            st = op_.tile([P, csb * P], BF16, tag="sel")
                            st3 = st[:].rearrange("p (g e) -> p g e", e=P)
                            nc.vector.tensor_tensor(
                                st3,
                                iota_s.unsqueeze(1)
                                .broadcast_to([P, csb, P]),
                                doff_s[:, cb0 : cb0 + csb]
                                .unsqueeze(2)
                                .broadcast_to([P, csb, P]),
                                mybir.AluOpType.is_equal,
                            )
                            nc.vector.tensor_tensor(
                                st3,
                                st3,
                                val_s[:, cb0 : cb0 + csb]
                                .unsqueeze(2)
                                .broadcast_to([P, csb, P]),
                                mybir.AluOpType.mult,
                            )
                            cc = 0
                            for wi, w in enumerate(ws):
                                for k in range(int(kwb[w, b])):
                                    first = win_seq[w][0] == (b, k)
                                    last = win_seq[w][-1] == (b, k)
                                    nc.tensor.matmul(
                                        out=wt[wi][:],
                                        lhsT=gb[:, cc * P : (cc + 1) * P],
                                        rhs=st[:, cc * P : (cc + 1) * P],
                                        start=first,
                                        stop=last,
                                        skip_group_check=True,
                                    )
                                    cc += 1

                        # weight stage for this SG
                        for wi, w in enumerate(ws):
                            meanT_sb = wp.tile([P, P], F32, tag="meanT")
                            nc.scalar.activation(
                                meanT_sb[:], wt[wi][:],
                                mybir.ActivationFunctionType.Identity,
                            )
                            zps = wpp.tile([P, P], F32, tag="zps", space="PSUM",
                                           bufs=1)
                            nc.tensor.matmul(
                                out=zps[:], lhsT=wl, rhs=meanT_sb[:],
                                start=True, stop=False,
                            )
                            nc.tensor.matmul(
                                out=zps[:], lhsT=wr,
                                rhs=rootT[:, w * P : (w + 1) * P],
                                start=False, stop=True,
                            )
                            if layer == 1:
                                # resident transposed h (+relu +bias), bf16
                                nc.scalar.activation(
                                    hT_s[:, w * P : (w + 1) * P], zps[:],
                                    mybir.ActivationFunctionType.Relu,
                                    bias=bl,
                                )
                                tps = wpp.tile([P, P], BF16, tag="tps",
                                               space="PSUM")
                                nc.tensor.transpose(
                                    out=tps[:],
                                    in_=hT_s[:, w * P : (w + 1) * P],
                                    identity=ident_bf[:],
                                )
                                hpub_sb = wp.tile([P, P], BF16, tag="hpub_sb")
                                nc.scalar.activation(
                                    hpub_sb[:], tps[:],
                                    mybir.ActivationFunctionType.Identity,
                                )
                                q, wq = w // cfg.WQ, w % cfg.WQ
                                nc.sync.dma_start(
                                    hpub[q][wq * P : (wq + 1) * P, :], hpub_sb[:]
                                )
                            else:
                                oT_sb = wp.tile([P, P], BF16, tag="oT_sb")
                                nc.scalar.activation(
                                    oT_sb[:], zps[:],
                                    mybir.ActivationFunctionType.Identity,
                                    bias=bl,
                                )
                                tps2 = wpp.tile([P, P], BF16, tag="tps",
                                                space="PSUM")
                                nc.tensor.transpose(
                                    out=tps2[:], in_=oT_sb[:],
                                    identity=ident_bf[:],
                                )
                                o_sb = wp.tile([P, P], BF16, tag="o_sb")
                                nc.scalar.activation(
                                    o_sb[:], tps2[:],
                                    mybir.ActivationFunctionType.Identity,
                                )
                                nc.sync.dma_start(
                                    out_t[w * P : (w + 1) * P, :], o_sb[:]
                                )

                        if layer == 1 and (s + 1) % (cfg.WQ // cfg.SGW) == 0:
                            q = (s + 1) // (cfg.WQ // cfg.SGW) - 1
                            nc.gpsimd.collective_compute(
                                "AllGather",
                                mybir.AluOpType.bypass,
                                replica_groups=[list(range(NCORES))],
                                ins=[hpub[q][:].opt()],
                                outs=[htbl[q][:].opt()],
                            )
    nc.finalize()
    return nc


def pack_inputs(cfg, prep, W_l1, b_l1, W_r1, W_l2, b_l2, W_r2):
    """Build the single packed bf16 input array per core."""
    nch = prep["nch"]
    nch2 = nch + (nch & 1)
    R0 = NQ * cfg.QP
    rows_total = R0 + nch + nch2 + 384 + 516

    iota = np.tile(np.arange(P, dtype=np.float32), (P, 1)).astype(BF)
    wbf = np.concatenate(
        [
            np.asarray(W_r1, np.float32).astype(BF),
            np.asarray(W_r2, np.float32).astype(BF),
            iota,
        ],
        axis=1,
    )  # [128, 384] bf16
    wf = np.concatenate(
        [
            np.asarray(W_l1, np.float32),
            np.asarray(W_l2, np.float32),
            np.asarray(b_l1, np.float32).reshape(D, 1),
            np.asarray(b_l2, np.float32).reshape(D, 1),
        ],
        axis=1,
    ).astype(np.float32)  # [128, 258] f32
    wbf_rows = np.ascontiguousarray(wbf).reshape(-1).reshape(384, P)
    wf_rows = (
        np.ascontiguousarray(wf).view(np.uint16).view(BF).reshape(-1)
        .reshape(516, P)
    )

    in_maps = []
    for c in range(NCORES):
        ext = np.zeros((rows_total, P), dtype=BF)
        ext[0:R0] = prep["xsh"][c]
        ext[R0 : R0 + nch] = (
            np.ascontiguousarray(prep["idxw16"][c]).view(BF).reshape(nch, P)
        )
        d8 = np.full((P, nch2), -1, dtype=np.int8)
        d8[:, :nch] = prep["doff8"][c]
        ext[R0 + nch : R0 + nch + nch2 // 2] = (
            np.ascontiguousarray(d8).view(BF).reshape(nch2 // 2, P)
        )
        c8 = np.ones((P, nch2), dtype=np.int8)
        c8[:, :nch] = prep["cnt8"][c]
        ext[R0 + nch + nch2 // 2 : R0 + nch + nch2] = (
            np.ascontiguousarray(c8).view(BF).reshape(nch2 // 2, P)
        )
        ext[R0 + nch + nch2 : R0 + nch + nch2 + 384] = wbf_rows
        ext[R0 + nch + nch2 + 384 :] = wf_rows
        in_maps.append(dict(xmy=ext))
    return in_maps


def kernel(x, edge_index, W_l1, b_l1, W_r1, W_l2, b_l2, W_r2):
    x = np.asarray(x, dtype=np.float32)
    cfg = Cfg(x.shape[0], np.asarray(edge_index).shape[1])
    prep = _host_prep(cfg, x, edge_index)
    in_maps = pack_inputs(cfg, prep, W_l1, b_l1, W_r1, W_l2, b_l2, W_r2)

    nc = _build_program(cfg, prep["kwb"], prep["nch"])
    res = run_bass_kernel_spmd(nc, in_maps, list(range(NCORES)))
    global LAST_EXEC_NS, LAST_RESULTS, LAST_NC, LAST_IN_MAPS
    LAST_EXEC_NS = res.exec_time_ns
    LAST_RESULTS = res
    LAST_NC = nc
    LAST_IN_MAPS = in_maps

    out = np.empty((cfg.N, D), dtype=np.float32)
    nodes = np.arange(cfg.N, dtype=np.int64)
    c_all = nodes // cfg.NSH
    local = nodes % cfg.NSH
    q_all = np.minimum(local // cfg.QR, NQ - 1)
    r_all = local - q_all * cfg.QR
    for c in range(NCORES):
        m = c_all == c
        out[nodes[m]] = (
            res.results[c]["out"].astype(np.float32)[(q_all * cfg.QP + r_all)[m]]
        )
    return out
